# revision 12
# baseline (speedup 1.0000x reference)
"""Contrastive loss kernel for Trainium2 (8 NeuronCores, SPMD data-parallel).

Problem: embedding [8192, 512] f32, label [8192] int64 (1024 classes).
    sim = E @ E.T
    loss = [ sum_{same,sim<1} (1-sim) + sum_{diff,sim>0.5} sim ] / n

Strategy
--------
Host: sort rows by label (the loss is permutation-invariant), downcast +
transpose the embedding to ET = E_sorted.T in fp8-e4m3. After sorting,
same-label pairs live within +-(max class size) of the diagonal, so the
label-dependent part of the loss only needs a narrow diagonal band.

Device (per core c, identical SPMD program, per-core input data):
  rows [1024c, 1024c+1024) of the sim matrix, all 8192 columns, streamed
  as 128x512 PSUM tiles. Matmuls run fp8 with perf_mode=DoubleRow (the
  128x128 PE array virtualizes to 128x256, 2x contraction per pass), so
  each 512-deep dot product takes 2 matmuls instead of 4.
  Main term for every pair:  f(s) = s * [s > 0.5]
                                  = relu(s - 0.5) + 0.5 * [s > 0.5]
    relu-sum: VectorE scalar_tensor_tensor (s-0.5) max 0 with fused
      accumulate, writing bf16 relu tiles into a [128, 4096] staging
      buffer (8 tiles per column chunk).
    count:    ScalarE Sign passes over 2048-wide slabs of the staging
      buffer with fused accumulate (relu >= 0, so sum(sign) == count,
      exact) - batching amortizes per-op overhead and keeps the whole
      count off the busier VectorE.
  Correction on the diagonal band (eq = same-label, via a small extra
  matmul of shipped window columns):  corr = eq * (relu(1-s) - f(s)).
  Host combines partials in float64:  loss = (sum f + sum corr) / n.

fp8 error analysis: products of e4m3 values are exact in the fp32 PSUM
accumulate; per-sim error std is ~2 for sim std 22.6. Sign-symmetric
threshold flips and value noise mostly cancel in the ~6e8-magnitude sum;
measured end-to-end relative error ~1e-5. The diagonal (sim_ii ~ 512,
excluded by the reference's sim<1 condition) cancels exactly because the
main-sweep and window matmuls compute bitwise-identical values.
"""

import numpy as np
import ml_dtypes

import concourse.bass as bass
import concourse.bacc as bacc
import concourse.tile as tile
from concourse import mybir
from concourse.bass_utils import run_bass_kernel_spmd

DT = mybir.dt
AT = mybir.ActivationFunctionType
OP = mybir.AluOpType

N = 8192          # rows
D = 512           # embedding dim
NCORES = 8
ROWS_PER_CORE = N // NCORES          # 1024
MT = ROWS_PER_CORE // 128            # 8 row-tiles of 128 per core
NJ = N // 512                        # 16 column chunks of 512
W = 256                              # diagonal-band window width
MAX_CLASS = 65                       # window correctness bound
MARGIN = 0.5
N_WARM = 10                          # dummy matmuls to trip the HAM warm-up

# acc layout (columns of the [128, 160] output):
#   [0:128)    sum relu(s-0.5)   per (j, m) chunk   (col = j*8+m)
#   [128:160)  count s>0.5       per (j, half) slab (col = 128+2j+h)
#   [160:168)  corr_pos = sum eq*relu(1-s)   per m
#   [168:176)  corr_neg = sum eq*f(s)        per m
ACC_COLS = 176

_CACHE = {}


def _build_program():
    """Build + compile the SPMD Bass program (same NEFF for all 8 cores)."""
    nc = bacc.Bacc("TRN2", target_bir_lowering=False, debug=False)

    # k-tile index = 2*t + i; DoubleRow matmul t contracts i=0,1 in one pass
    rhs_d = nc.dram_tensor("rhs", (2, 2, 128, N), DT.float8e4, kind="ExternalInput")
    lhsT_d = nc.dram_tensor("lhsT", (2, 2, 128, ROWS_PER_CORE), DT.float8e4,
                            kind="ExternalInput")
    win_d = nc.dram_tensor("win", (MT, 2, 2, 128, W), DT.float8e4,
                           kind="ExternalInput")
    labw_d = nc.dram_tensor("labw", (MT, 128, W), DT.float16,
                            kind="ExternalInput")
    labo_d = nc.dram_tensor("labo", (128, MT), DT.float32, kind="ExternalInput")
    accs_d = nc.dram_tensor("accs", (128, ACC_COLS), DT.float32,
                            kind="ExternalOutput")

    DR = mybir.MatmulPerfMode.DoubleRow

    with tile.TileContext(nc) as tc:
        with (
            tc.tile_pool(name="const", bufs=1) as constp,
            tc.tile_pool(name="rhsp", bufs=3) as rhsp,
            tc.tile_pool(name="sap", bufs=2) as sap,
            tc.tile_pool(name="scr", bufs=2) as scrp,
            tc.tile_pool(name="wscr", bufs=2) as wscrp,
            tc.tile_pool(name="psum", bufs=6, space=bass.MemorySpace.PSUM) as psp,
            tc.tile_pool(name="wpsum", bufs=2, space=bass.MemorySpace.PSUM) as wpsp,
        ):
            # --- PE warm-up: dummy matmuls with no input dependencies ----
            dummy = constp.tile([128, 512], DT.bfloat16, tag="dummy")
            nc.gpsimd.memset(dummy[:], 0.0)
            for w in range(N_WARM):
                wps = wpsp.tile([128, 512], DT.float32, tag="wmm")
                nc.tensor.matmul(wps[:], dummy[:, 0:128], dummy[:],
                                 start=True, stop=True)

            # --- constants -----------------------------------------------
            zeros = constp.tile([128, 512], DT.bfloat16, tag="zeros")
            nc.vector.memset(zeros[:], 0.0)
            acc = constp.tile([128, ACC_COLS], DT.float32, tag="acc")

            # --- per-core data (all DMAs on the sync queue, stream order) -
            rt0 = rhsp.tile([128, 2, 2, 512], DT.float8e4, tag="rhs")
            nc.sync.dma_start(rt0[:],
                              rhs_d[:, :, :, 0:512].rearrange("t i p n -> p t i n"))
            lhsT_sb = constp.tile([128, 2, 2, ROWS_PER_CORE], DT.float8e4,
                                  tag="lhsT")
            nc.sync.dma_start(lhsT_sb[:],
                              lhsT_d[:].rearrange("t i p m -> p t i m"))
            labw_sb = constp.tile([128, MT, W], DT.float16, tag="labw")
            nc.sync.dma_start(labw_sb[:], labw_d[:].rearrange("m p w -> p m w"))
            labo_sb = constp.tile([128, MT], DT.float32, tag="labo")
            nc.sync.dma_start(labo_sb[:], labo_d[:])
            win_sb = constp.tile([128, MT, 2, 2, W], DT.float8e4, tag="win")
            nc.sync.dma_start(win_sb[:, 0],
                              win_d[0].rearrange("t i p w -> p t i w"))

            for j in range(NJ):
                if j > 0:
                    rt = rhsp.tile([128, 2, 2, 512], DT.float8e4, tag="rhs")
                    nc.sync.dma_start(
                        rt[:], rhs_d[:, :, :, j * 512:(j + 1) * 512]
                        .rearrange("t i p n -> p t i n"))
                else:
                    rt = rt0
                if j < MT - 1:  # prefetch next window columns
                    nc.sync.dma_start(
                        win_sb[:, j + 1],
                        win_d[j + 1].rearrange("t i p w -> p t i w"))

                sa = sap.tile([128, MT * 512], DT.bfloat16, tag="sa")
                for m in range(MT):
                    ps = psp.tile([128, 512], DT.float32, tag="mm")
                    for t in range(2):
                        nc.tensor.matmul(
                            ps[:], lhsT_sb[:, t, :, m * 128:(m + 1) * 128],
                            rt[:, t, :, :], start=(t == 0), stop=(t == 1),
                            perf_mode=DR)
                    slot = j * MT + m
                    sam = sa[:, m * 512:(m + 1) * 512]
                    # VectorE: sa = (s - 0.5) max 0, fused accum -> sum
                    nc.vector.scalar_tensor_tensor(
                        sam, ps[:], MARGIN, zeros[:], op0=OP.subtract,
                        op1=OP.max, accum_out=acc[:, slot:slot + 1])
                # ScalarE: count via sign over 4-tile relu slabs
                # (exact: sa >= 0, so sum(sign(sa)) == #(sa > 0) == #(s > 0.5))
                for h in range(2):
                    sg = scrp.tile([128, 2048], DT.bfloat16, tag="sg")
                    cslot = 128 + 2 * j + h
                    nc.scalar.activation(sg[:], sa[:, h * 2048:(h + 1) * 2048],
                                         AT.Sign,
                                         accum_out=acc[:, cslot:cslot + 1])

                if j < MT:
                    m = j
                    wp = wpsp.tile([128, W], DT.float32, tag="wmm")
                    for t in range(2):
                        nc.tensor.matmul(
                            wp[:], lhsT_sb[:, t, :, m * 128:(m + 1) * 128],
                            win_sb[:, m, t, :, :], start=(t == 0), stop=(t == 1),
                            perf_mode=DR)
                    # eq = [label_col == label_row]  {0,1}
                    eq_t = wscrp.tile([128, W], DT.bfloat16, tag="eq")
                    nc.vector.tensor_scalar(eq_t[:], labw_sb[:, m, :],
                                            labo_sb[:, m:m + 1], None,
                                            op0=OP.is_equal)
                    # g = relu(1 - s)   (ScalarE)
                    g_t = wscrp.tile([128, W], DT.bfloat16, tag="g")
                    nc.scalar.activation(g_t[:], wp[:], AT.Relu, bias=1.0, scale=-1.0)
                    # es = eq * s
                    es_t = wscrp.tile([128, W], DT.float32, tag="es")
                    nc.vector.tensor_tensor(es_t[:], eq_t[:], wp[:], op=OP.mult)
                    # corr_neg += sum (s > 0.5) * (eq * s)
                    w1 = wscrp.tile([128, W], DT.float32, tag="w1")
                    nc.vector.scalar_tensor_tensor(
                        w1[:], wp[:], MARGIN, es_t[:], op0=OP.is_gt, op1=OP.mult,
                        accum_out=acc[:, 168 + m:169 + m])
                    # corr_pos += sum eq * relu(1 - s)
                    w2 = wscrp.tile([128, W], DT.bfloat16, tag="w2")
                    nc.vector.scalar_tensor_tensor(
                        w2[:], eq_t[:], 1.0, g_t[:], op0=OP.mult, op1=OP.mult,
                        accum_out=acc[:, 160 + m:161 + m])

            nc.sync.dma_start(accs_d[:], acc[:])

    nc.compile()
    return nc


def _host_prep(embedding, label):
    """Sort by label, build per-core input maps."""
    embedding = np.asarray(embedding, dtype=np.float32)
    label = np.asarray(label).astype(np.int64)
    perm = np.argsort(label, kind="stable")
    labels_s = label[perm]
    Es = embedding[perm]

    cls_max = int(np.bincount(labels_s).max())
    if cls_max > MAX_CLASS:
        return None  # caller falls back to numpy path

    ET = np.ascontiguousarray(Es.T).astype(ml_dtypes.float8_e4m3)  # [D, N]
    ET4 = ET.reshape(2, 2, 128, N)   # [t, i, p, col]; k-tile = 2t + i

    labf = labels_s.astype(np.float16)                            # exact (< 2048)
    in_maps = []
    for c in range(NCORES):
        r0 = c * ROWS_PER_CORE
        lhsT = np.ascontiguousarray(ET4[:, :, :, r0:r0 + ROWS_PER_CORE])

        win = np.zeros((MT, 2, 2, 128, W), dtype=ml_dtypes.float8_e4m3)
        labw = np.full((MT, W), -1.0, dtype=np.float16)
        for m in range(MT):
            T = c * MT + m
            lo = 128 * T - 64
            a = max(lo, 0)
            b = min(lo + W, N)
            win[m, :, :, :, a - lo:b - lo] = ET4[:, :, :, a:b]
            labw[m, a - lo:b - lo] = labf[a:b]
        labw_b = np.ascontiguousarray(
            np.broadcast_to(labw[:, None, :], (MT, 128, W))).astype(np.float16)

        labo = np.ascontiguousarray(
            labels_s[r0:r0 + ROWS_PER_CORE].reshape(MT, 128).T
        ).astype(np.float32)

        in_maps.append({
            "rhs": ET4,
            "lhsT": lhsT,
            "win": win,
            "labw": labw_b,
            "labo": labo,
        })
    return in_maps


def _reduce_accs(results):
    """Combine per-core [128, 160] partials into the scalar loss (float64)."""
    total = 0.0
    for res in results:
        a = res["accs"].astype(np.float64)
        s_relu = a[:, 0:128].sum()
        s_cnt = a[:, 128:160].sum()
        c_pos = a[:, 160:168].sum()
        c_neg = a[:, 168:176].sum()
        total += s_relu + MARGIN * s_cnt + c_pos - c_neg
    return total / N


def _numpy_fallback(embedding, label):
    emb = np.asarray(embedding, dtype=np.float32)
    lab = np.asarray(label)
    sim = emb @ emb.T
    same = lab[:, None] == lab[None, :]
    pos = np.where(same & (sim < 1.0), 1.0 - sim, 0.0).sum(dtype=np.float64)
    neg = np.where((~same) & (sim > MARGIN), sim, 0.0).sum(dtype=np.float64)
    return (pos + neg) / emb.shape[0]


def _run(embedding, label, trace=False):
    if "nc" not in _CACHE:
        _CACHE["nc"] = _build_program()
    nc = _CACHE["nc"]

    in_maps = _host_prep(embedding, label)
    if in_maps is None:
        return _numpy_fallback(embedding, label), None

    res = run_bass_kernel_spmd(nc, in_maps, core_ids=list(range(NCORES)),
                               trace=trace)
    loss = _reduce_accs(res.results)
    return loss, res


def kernel(embedding, label):
    assert embedding.shape == (N, D), embedding.shape
    assert label.shape == (N,), label.shape
    loss, _ = _run(embedding, label, trace=False)
    return (np.float32(loss), 0, 0)
